# revision 1
# baseline (speedup 1.0000x reference)
"""Dual-modality (opt/sar) multiplicative cross-attention — TRN2 Bass kernel.

Reference computation (per sample n, C=64, HW=64*64=4096):
  q_m = W_q^m x + b_q^m ; k_m = W_k^m x + b_k^m ; v_m = W_v^m x + b_v^m   (m in {opt,sar})
  att = softmax(q_o k_o^T) * softmax(q_s k_s^T)        (elementwise, (HW,HW))
  out = (att @ v_o) * (att @ v_s)                      -> (C,H,W) layout

Restructured for the hardware (v2):
  S_m[i,j] = q_i.k_j = x_i^T M_m x_j + u_m.x_i + w_m.x_j + b0_m,
    M = Wq^T Wk, u = Wq^T bk, w = Wk^T bq, b0 = bq.bk  (host-precomputed).
  The row-constant term u.x_i cancels exactly in the softmax normalization
  (it scales row i of A_m and sums_m identically), so it is dropped:
    S~[i,j] = [x_i; 1] . Y_m[:, j],  Y_m = Ly_m^T [x; 1],  contraction 65.
  A_m = exp(S~_m - SHIFT)  (ACT, accum_out -> row sums)
  P = A_o * A_s  (one DVE tensor_tensor per q-block)
  P^T via xbar DMA transpose (slot order is some fixed bijection of columns;
  v_pair = [v_o; v_s] (128, HW) goes through the IDENTICAL transpose geometry,
  so the U contraction pairs matching k regardless of the internal slot map).
  U[q, 0:64|64:128] = sum_k P^T[k,q] v_pair^T[k, :]   (32 PE matmuls, PSUM)
  out[q, c] = U[:,0:64]*U[:,64:128] * (sums_o*sums_s)^-2   -> DRAM (HALF, C)
  Host transposes each core's (HALF, C) block back to (C, HALF).

Sharding: 8 cores, core c handles sample c//2, query-row half c%2 (2048 rows).
The core's x is column-rotated on host so its q-half is always columns 0:2048
(k-column order is permutation-invariant end-to-end).

Dtypes: S matmuls float32r; A/P/v bf16; sums/denoms/U fp32.
"""
import numpy as np
from contextlib import ExitStack

import concourse.bass as bass
import concourse.tile as tile
from concourse import bacc, mybir
from concourse import masks
from concourse.bass_utils import run_bass_kernel_spmd

N, C, H, W = 4, 64, 64, 64
HW = H * W            # 4096
HALF = HW // 2        # 2048 query rows per core
NBLK = HALF // 128    # 16 q-blocks per core
SHIFT = 30.0
CE = C + 1            # 65: x rows + ones row

# exp chunking of the 4096 k-columns of one (modality, q-block) unit;
# each chunk is one PSUM tile fill + one ACT exp call with accum.
CHUNKS = [(0, 1536), (1536, 1536), (3072, 1024)]

dt = mybir.dt
AF = mybir.ActivationFunctionType

_compiled = None


def _build(repeat=1, dbg=False):
    nc = bacc.Bacc("TRN2", debug=False)
    d_in = {}
    for m in ("opt", "sar"):
        d_in[f"x_{m}"] = nc.dram_tensor(f"x_{m}", (CE, HW), dt.float32r, kind="ExternalInput").ap()
        d_in[f"Ly_{m}"] = nc.dram_tensor(f"Ly_{m}", (CE, CE), dt.float32r, kind="ExternalInput").ap()
        d_in[f"Lv_{m}"] = nc.dram_tensor(f"Lv_{m}", (CE, C), dt.float32r, kind="ExternalInput").ap()
    d_out = nc.dram_tensor("out", (HALF, C), dt.float32, kind="ExternalOutput").ap()

    with tile.TileContext(nc) as tc, ExitStack() as ctx:
        consts = ctx.enter_context(tc.tile_pool(name="consts", bufs=1))
        xext = ctx.enter_context(tc.tile_pool(name="xext", bufs=1))
        proj = ctx.enter_context(tc.tile_pool(name="proj", bufs=1))
        work = ctx.enter_context(tc.tile_pool(name="work", bufs=3))
        workP = ctx.enter_context(tc.tile_pool(name="workP", bufs=3))
        stats = ctx.enter_context(tc.tile_pool(name="stats", bufs=2))
        ps_S = ctx.enter_context(tc.tile_pool(name="ps_S", bufs=2, space="PSUM"))
        ps_T = ctx.enter_context(tc.tile_pool(name="ps_T", bufs=1, space="PSUM"))
        ps_U = ctx.enter_context(tc.tile_pool(name="ps_U", bufs=1, space="PSUM"))

        # ---- consts; ACT exp-table warm-up rides the first activation ----
        neg_shift = consts.tile([128, 1], dt.float32)
        nc.gpsimd.memset(neg_shift[:], -SHIFT)
        ident_bf = consts.tile([128, 128], dt.bfloat16)
        masks.make_identity(nc, ident_bf[:])
        warm = consts.tile([128, 1], dt.float32)
        nc.scalar.activation(warm[:], neg_shift[:], AF.Exp)

        # ---- input DMAs: opt weights+x first so Y_opt projection starts asap
        # single DMA queue, ordered by need: x_opt chunk 1 first so the
        # Y_opt projection (and the first exp) start as early as possible.
        Ly = {}
        Lv = {}
        x_ext = {}
        for m in ("opt", "sar"):
            x_ext[m] = xext.tile([CE, HW], dt.float32r, tag=f"xe_{m}", name=f"xe_{m}")
            Ly[m] = consts.tile([CE, CE], dt.float32r, tag=f"Ly_{m}", name=f"ly_{m}")
            Lv[m] = consts.tile([CE, C], dt.float32r, tag=f"Lv_{m}", name=f"lv_{m}")
        for m in ("opt", "sar"):
            nc.sync.dma_start(x_ext[m][:, bass.ts(0, HW // 4)],
                              d_in[f"x_{m}"][:, bass.ts(0, HW // 4)])
            nc.sync.dma_start(Ly[m][:], d_in[f"Ly_{m}"][:])
            for dc in range(1, 4):
                nc.sync.dma_start(x_ext[m][:, bass.ts(dc, HW // 4)],
                                  d_in[f"x_{m}"][:, bass.ts(dc, HW // 4)])
        for m in ("opt", "sar"):
            nc.sync.dma_start(Lv[m][:], d_in[f"Lv_{m}"][:])
        # PE pstate warm-up: keep PE busy until real matmuls arrive
        pewarm = ps_T.tile([128, 128], dt.bfloat16, tag="T", name="pewarm",
                           padded_shape=[128, 1024])
        for _w in range(12):
            nc.tensor.transpose(pewarm[:, 0:128], ident_bf[:], ident_bf[:])

        Y = {m: proj.tile([CE, HW], dt.float32r, tag=f"Y_{m}", name=f"Y_{m}")
             for m in ("opt", "sar")}
        v_pair = proj.tile([128, HW], dt.bfloat16)
        vT = proj.tile([128, HW // 128, 128], dt.bfloat16)

        def emit_proj_chunk(dst, lhsT, m, rowoff, nrows, cstart, clen, split=False,
                            act_copy=False):
            ps = ps_S.tile([128, 1536], dt.float32, tag="S", name="psp")
            for j in range(clen // 512):
                nc.tensor.matmul(ps[0:nrows, bass.ts(j, 512)], lhsT[:],
                                 x_ext[m][:, cstart + 512 * j:cstart + 512 * (j + 1)],
                                 start=True, stop=True)
                if split:
                    nc.vector.tensor_copy(
                        dst[rowoff:rowoff + nrows, cstart + 512 * j:cstart + 512 * (j + 1)],
                        ps[0:nrows, 512 * j:512 * (j + 1)])
            if not split:
                if act_copy:
                    # ACT is idle pre-first-exp; copying on it removes the
                    # cross-engine handoff before the exp that consumes dst.
                    nc.scalar.copy(dst[rowoff:rowoff + nrows, cstart:cstart + clen],
                                   ps[0:nrows, 0:clen])
                else:
                    nc.vector.tensor_copy(dst[rowoff:rowoff + nrows, cstart:cstart + clen],
                                          ps[0:nrows, 0:clen])

        def emit_S_exp_chunk(i, m, A, pt, ci):
            cstart, clen = CHUNKS[ci]
            ps = ps_S.tile([128, 1536], dt.float32, tag="S", name="pss")
            for j in range(clen // 512):
                nc.tensor.matmul(ps[:, bass.ts(j, 512)],
                                 x_ext[m][:, bass.ts(i, 128)],
                                 Y[m][:, cstart + 512 * j:cstart + 512 * (j + 1)],
                                 start=True, stop=True)
            nc.scalar.activation(A[:, cstart:cstart + clen], ps[:, 0:clen],
                                 AF.Exp, bias=neg_shift[:],
                                 accum_out=pt[:, ci:ci + 1])

        def emit_S_exp(i, m, A, pt):
            for ci in range(len(CHUNKS)):
                emit_S_exp_chunk(i, m, A, pt, ci)

        def alloc_block(i):
            A_o = work.tile([128, HW], dt.bfloat16, tag="A_opt", name="A_o")
            pt_o = stats.tile([128, len(CHUNKS)], dt.float32, tag="pt_o", name="pt_o")
            A_s = work.tile([128, HW], dt.bfloat16, tag="A_sar", name="A_s")
            pt_s = stats.tile([128, len(CHUNKS)], dt.float32, tag="pt_s", name="pt_s")
            return A_o, A_s, pt_o, pt_s

        def emit_block_S(i):
            blk = alloc_block(i)
            emit_S_exp(i, "opt", blk[0], blk[2])
            emit_S_exp(i, "sar", blk[1], blk[3])
            return blk

        # transpose group g (1024 P-cols) is ready once exp chunk CGRP[g] done
        CGRP = [0, 1, 1, 2]

        def emit_P(A_o, A_s):
            # P in exp-chunk-aligned pieces so transposes can start early
            P = workP.tile([128, HW], dt.bfloat16, tag="P")
            for cstart, clen in CHUNKS:
                nc.vector.tensor_mul(P[:, cstart:cstart + clen],
                                     A_o[:, cstart:cstart + clen],
                                     A_s[:, cstart:cstart + clen])
            return P

        def emit_rest(i, A_o, A_s, pt_o, pt_s, tail=False, pre_P=None):
            P = pre_P if pre_P is not None else emit_P(A_o, A_s)
            PT = workP.tile([128, HW // 128, 128], dt.bfloat16, tag="PT")
            # denominators first: they only need the accum partials, and
            # doing them now keeps the post-U critical path to just t0/ob.
            sums_o = stats.tile([128, 1], dt.float32, tag="sums_o")
            sums_s = stats.tile([128, 1], dt.float32, tag="sums_s")
            nc.vector.reduce_sum(sums_o[:], pt_o[:], axis=mybir.AxisListType.X)
            nc.vector.reduce_sum(sums_s[:], pt_s[:], axis=mybir.AxisListType.X)
            denom = stats.tile([128, 1], dt.float32, tag="denom")
            nc.vector.tensor_mul(denom[:], sums_o[:], sums_s[:])
            inv = stats.tile([128, 1], dt.float32, tag="inv")
            nc.vector.reciprocal(inv[:], denom[:])
            inv2 = stats.tile([128, 1], dt.float32, tag="inv2")
            nc.vector.tensor_mul(inv2[:], inv[:], inv[:])

            U = ps_U.tile([128, 128], dt.float32, tag="U")
            for g in range(HW // 1024):
                if tail and g % 2 == 1:
                    # last block: odd groups stage through the ps_S banks
                    # (free after the final exps) to break the serial
                    # transpose->copy chain through the single ps_T bank.
                    pst = ps_S.tile([128, 1024], dt.bfloat16, tag="S", name="pstS")
                else:
                    pst = ps_T.tile([128, 1024], dt.bfloat16, tag="T", name="pst")
                for t in range(8):
                    b = g * 8 + t
                    nc.tensor.transpose(pst[:, bass.ts(t, 128)],
                                        P[:, bass.ts(b, 128)], ident_bf[:])
                nc.vector.tensor_copy(PT[:, g * 8:(g + 1) * 8, :], pst[:])
                if tail:
                    for t in range(8):
                        b = g * 8 + t
                        nc.tensor.matmul(U[:], PT[:, b, :], vT[:, b, :],
                                         start=(b == 0), stop=(b == HW // 128 - 1))
            if not tail:
                for b in range(HW // 128):
                    nc.tensor.matmul(U[:], PT[:, b, :], vT[:, b, :],
                                     start=(b == 0), stop=(b == HW // 128 - 1))

            t0 = stats.tile([128, C], dt.float32, tag="t0")
            nc.vector.tensor_scalar_mul(t0[:], U[:, 0:C], inv2[:])
            ob = stats.tile([128, C], dt.float32, tag="ob")
            nc.vector.tensor_mul(ob[:], t0[:], U[:, C:128])
            nc.sync.dma_start(d_out[i * 128:(i + 1) * 128, :], ob[:])

        # ---- startup: interleave Y projections with block 0 per chunk ----
        blk0 = alloc_block(0)
        for ci, (cstart, clen) in enumerate(CHUNKS):
            emit_proj_chunk(Y["opt"], Ly["opt"], "opt", 0, CE, cstart, clen, split=(ci == 0))
            emit_proj_chunk(Y["sar"], Ly["sar"], "sar", 0, CE, cstart, clen, split=(ci == 0))
            emit_S_exp_chunk(0, "opt", blk0[0], blk0[2], ci)
            emit_S_exp_chunk(0, "sar", blk0[1], blk0[3], ci)
        blk1 = emit_block_S(1)
        P0 = emit_P(blk0[0], blk0[1])

        # v projections + vT transposes on the ps_T bank (do not touch ps_S)
        for mi, m in enumerate(("opt", "sar")):
            for jc in range(HW // 512):
                # alternate staging between the ps_T and ps_U banks: ps_U's
                # first real consumer (U of block 0) strictly follows vT, so
                # borrowing it here is safe and halves the serial chain.
                if jc % 2 == 0:
                    psv = ps_T.tile([128, 512], dt.float32, tag="T", name="psv",
                                    padded_shape=[128, 512])
                else:
                    psv = ps_U.tile([128, 512], dt.float32, tag="U", name="psvU",
                                    padded_shape=[128, 512])
                nc.tensor.matmul(psv[0:C, :], Lv[m][:],
                                 x_ext[m][:, bass.ts(jc, 512)],
                                 start=True, stop=True)
                nc.vector.tensor_copy(v_pair[64 * mi:64 * mi + C, bass.ts(jc, 512)],
                                      psv[0:C, :])
        for g in range(HW // 1024):
            pst = ps_T.tile([128, 1024], dt.bfloat16, tag="T", name="pstv")
            for t in range(8):
                b = g * 8 + t
                nc.tensor.transpose(pst[:, bass.ts(t, 128)],
                                    v_pair[:, bass.ts(b, 128)], ident_bf[:])
            nc.vector.tensor_copy(vT[:, g * 8:(g + 1) * 8, :], pst[:])

        pend = blk0
        nxt = blk1
        for i in range(NBLK * repeat):
            ii = i % NBLK
            emit_rest(ii, *pend, tail=(i == NBLK * repeat - 1),
                      pre_P=(P0 if i == 0 else None))
            pend = nxt
            j = i + 1
            nxt = emit_block_S((j + 1) % NBLK) if j + 1 < NBLK * repeat else None
            # note: emit order gives S of block j+1 priority over rest of j

    nc.compile()
    return nc


def _to_f32r(x):
    """Round fp32 to the float32r format: RNE to 11 mantissa bits, low 12 bits zero."""
    u = np.ascontiguousarray(x, np.float32).view(np.uint32)
    lsb = (u >> 12) & 1
    r = (u + np.uint32(0x7FF) + lsb) & np.uint32(0xFFFFF000)
    return r.view(np.float32)


def kernel(x_opt, x_sar, wq_opt, bq_opt, wk_opt, bk_opt, wv_opt, bv_opt,
           wq_sar, bq_sar, wk_sar, bk_sar, wv_sar, bv_sar, _trace=False):
    global _compiled
    if _compiled is None:
        _compiled = _build()
    nc = _compiled

    common = {}
    for m, wq, bq, wk, bk, wv, bv in (
            ("opt", wq_opt, bq_opt, wk_opt, bk_opt, wv_opt, bv_opt),
            ("sar", wq_sar, bq_sar, wk_sar, bk_sar, wv_sar, bv_sar)):
        wq = np.asarray(wq, np.float64); wk = np.asarray(wk, np.float64)
        bq = np.asarray(bq, np.float64); bk = np.asarray(bk, np.float64)
        M = wq.T @ wk                      # (64, 64)
        w_vec = wk.T @ bq                  # (64,)
        b0 = float(bq @ bk)
        Lym = np.zeros((CE, CE), np.float64)
        Lym[0:C, 0:C] = M.T
        Lym[0:C, C] = w_vec
        Lym[C, C] = b0
        common[f"Ly_{m}"] = _to_f32r(Lym.astype(np.float32))
        Lvm = np.zeros((CE, C), np.float64)
        Lvm[0:C, :] = np.asarray(wv, np.float64).T
        Lvm[C, :] = np.asarray(bv, np.float64)
        common[f"Lv_{m}"] = _to_f32r(Lvm.astype(np.float32))

    in_maps = []
    for core in range(8):
        n, h = core // 2, core % 2
        mdict = dict(common)
        for m, x in (("opt", x_opt), ("sar", x_sar)):
            xs = np.asarray(x[n], np.float32).reshape(C, HW)
            if h:
                xs = np.concatenate([xs[:, HALF:], xs[:, :HALF]], axis=1)
            xs = np.concatenate([xs, np.ones((1, HW), np.float32)], axis=0)
            mdict[f"x_{m}"] = _to_f32r(xs)
        in_maps.append(mdict)

    r = run_bass_kernel_spmd(nc, in_maps, core_ids=list(range(8)), trace=_trace)
    out = np.empty((N, C, HW), np.float32)
    for core in range(8):
        n, h = core // 2, core % 2
        out[n][:, h * HALF:(h + 1) * HALF] = r.results[core]["out"].T
    kernel._last_result = r
    return out.reshape(N, C, H, W)



# revision 3
# speedup vs baseline: 1.0353x; 1.0353x over previous
"""Dual-modality (opt/sar) multiplicative cross-attention — TRN2 Bass kernel v3.

Reference computation (per sample n, C=64, HW=64*64=4096):
  q_m = W_q^m x + b_q^m ; k_m = W_k^m x + b_k^m ; v_m = W_v^m x + b_v^m
  att = softmax(q_o k_o^T) * softmax(q_s k_s^T)        (elementwise, (HW,HW))
  out = (att @ v_o) * (att @ v_s)                      -> (C,H,W) layout

v3 restructure (vs v2 baseline at 167 us):
  S~_m[i,j] = x̃_i^T G_m x̃_j,  G = [[Wq^T Wk, 0],[ (Wk^T bq)^T, bq.bk - SHIFT ]]
  (the row-constant q-bias term cancels in softmax; SHIFT baked into G so
  exp needs no bias operand).
  Yalt_m = G_m^T x̃[:, q-window]  (65 x 2048): lhsT for the S matmuls; the
  rhs is raw x̃, so no (65 x 4096) k-side projection is materialized.
  A_m = exp(S_m) via ACT in 3 PSUM chunks (1536/1536/1024) -> bf16 SBUF.
  A^T: opt via DMA-crossbar transposes on the ACT queue (SP-queue issue
  races with the producer's SBUF write - measured), sar via PE transposes
  (c0, c1) + one late DMA (c2). Z_m = row sums via 32 accumulating PE
  matmuls against a ones vector (replaces ACT accum_out reads).
  P^T = A_o^T * A_s^T elementwise on DVE (bf16 4x mode).
  U[q, 0:64|64:128] = sum_b PT[:,b,:]^T v_pairT[:,b,:]  (32 PE matmuls).
  z_o/z_s/U share one PSUM bank; their accumulation chains are strictly
  sequential (interleaved chains in one bank corrupt - measured).
  out[q, c] = U[:,0:C]*U[:,C:]*(z_o*z_s)^-2 -> DRAM (HALF, C); host
  transposes. The out-DMA uses the SWDGE (Pool) path to keep HWDGE
  semaphore lanes transpose-only.

Scheduling: 1-block software pipeline, hand-interleaved so the in-order PE
stream (S fills pace with ACT via the 2-buf PSUM rotation) absorbs the
z/U chains and PE transposes without starving ACT. Tile's cross-queue
DMA-lane semaphores couple ACT progress to out-DMA completion within
~1.5 blocks, so the whole per-block chain must drain quickly - deeper
pipelining regresses.

Sharding: 8 cores, core c handles sample c//2, query-row half c%2 (2048
rows). The core's x is column-rotated on host so its q-half is always
columns 0:2048.
"""
import numpy as np
from contextlib import ExitStack

import concourse.bass as bass
import concourse.tile as tile
from concourse import bacc, mybir
from concourse import masks
from concourse.bass_utils import run_bass_kernel_spmd

N, C, H, W = 4, 64, 64, 64
HW = H * W            # 4096
HALF = HW // 2        # 2048 query rows per core
NBLK = HALF // 128    # 16 q-blocks per core
SHIFT = 30.0
CE = C + 1            # 65: x rows + ones row
# Schraudolph bf16-exp (DVE bit-trick) for the c2 chunks of both
# modalities: y = round(128*log2(e)*S + (127*128 - C7)); bitcast uint16
# -> bf16 approximates exp(S) within +-4.5%. C7 centers the one-sided
# linear-mantissa error; numerically validated end-to-end (rel ~3e-3).
SCHRAU_SCALE = 128 * 1.4426950408889634
SCHRAU_C7 = 9.0
SCHRAU_BIAS = 127 * 128 - SCHRAU_C7

CHUNKS = [(0, 1536), (1536, 1536), (3072, 1024)]

dt = mybir.dt
AF = mybir.ActivationFunctionType

_compiled = None


def _build(repeat=1):
    nc = bacc.Bacc("TRN2", debug=False)
    d_in = {}
    for m in ("opt", "sar"):
        d_in[f"x_{m}"] = nc.dram_tensor(f"x_{m}", (CE, HW), dt.float32r, kind="ExternalInput").ap()
        d_in[f"G_{m}"] = nc.dram_tensor(f"G_{m}", (CE, CE), dt.float32r, kind="ExternalInput").ap()
        d_in[f"Lv_{m}"] = nc.dram_tensor(f"Lv_{m}", (CE, C), dt.float32r, kind="ExternalInput").ap()
    d_out = nc.dram_tensor("out", (128, NBLK * C), dt.float32, kind="ExternalOutput").ap()

    with tile.TileContext(nc) as tc, ExitStack() as ctx:
        consts = ctx.enter_context(tc.tile_pool(name="consts", bufs=1))
        xext = ctx.enter_context(tc.tile_pool(name="xext", bufs=1))
        proj = ctx.enter_context(tc.tile_pool(name="proj", bufs=1))
        workA = ctx.enter_context(tc.tile_pool(name="workA", bufs=2))
        workT = ctx.enter_context(tc.tile_pool(name="workT", bufs=3))
        workP = ctx.enter_context(tc.tile_pool(name="workP", bufs=3))
        stats = ctx.enter_context(tc.tile_pool(name="stats", bufs=2))
        ps_S = ctx.enter_context(tc.tile_pool(name="ps_S", bufs=2, space="PSUM"))
        ps_T = ctx.enter_context(tc.tile_pool(name="ps_T", bufs=1, space="PSUM"))
        ps_U = ctx.enter_context(tc.tile_pool(name="ps_U", bufs=1, space="PSUM"))

        # ---- consts; ACT exp-table warm-up rides the first activation ----
        warmsrc = consts.tile([128, 1], dt.float32)
        nc.gpsimd.memset(warmsrc[:], -SHIFT)
        ones_bf = consts.tile([128, 1], dt.bfloat16)
        nc.gpsimd.memset(ones_bf[:], 1.0)
        ident_bf = consts.tile([128, 128], dt.bfloat16)
        masks.make_identity(nc, ident_bf[:])
        warm = consts.tile([128, 1], dt.float32)
        nc.scalar.activation(warm[:], warmsrc[:], AF.Exp)

        # ---- input DMAs, ordered by need ----
        G = {}
        Lv = {}
        x_ext = {}
        for m in ("opt", "sar"):
            x_ext[m] = xext.tile([CE, HW], dt.float32r, tag=f"xe_{m}", name=f"xe_{m}")
            G[m] = consts.tile([CE, CE], dt.float32r, tag=f"G_{m}", name=f"g_{m}")
            Lv[m] = consts.tile([CE, C], dt.float32r, tag=f"Lv_{m}", name=f"lv_{m}")
        def _xdma(m, lo, hi):
            nc.sync.dma_start(x_ext[m][:, lo:hi], d_in[f"x_{m}"][:, lo:hi])
        _xdma("opt", 0, 512)
        nc.sync.dma_start(G["opt"][:], d_in["G_opt"][:])
        _xdma("opt", 512, 1536)
        _xdma("sar", 0, 512)
        nc.sync.dma_start(G["sar"][:], d_in["G_sar"][:])
        _xdma("opt", 1536, 3072)
        _xdma("sar", 512, 1536)
        _xdma("opt", 3072, 4096)
        _xdma("sar", 1536, 3072)
        _xdma("sar", 3072, 4096)
        for m in ("opt", "sar"):
            nc.sync.dma_start(Lv[m][:], d_in[f"Lv_{m}"][:])

        # PE pstate warm-up: keep PE busy until real matmuls arrive
        pewarm = ps_T.tile([128, 128], dt.bfloat16, tag="T", name="pewarm",
                           padded_shape=[128, 1024])
        for _w in range(3):
            nc.tensor.transpose(pewarm[:, 0:128], ident_bf[:], ident_bf[:])

        # ---- Yalt projections: Yalt = G^T x̃[:, 0:HALF]  (65 x 2048) ----
        Yalt = {m: proj.tile([CE, HALF], dt.float32r, tag=f"Ya_{m}", name=f"Ya_{m}")
                for m in ("opt", "sar")}
        def emit_yalt(m, jc):
            ps = ps_S.tile([128, 1536], dt.float32, tag="S", name="psy")
            nc.tensor.matmul(ps[0:CE, 0:512], G[m][:],
                             x_ext[m][:, 512 * jc:512 * (jc + 1)],
                             start=True, stop=True)
            if m == "opt":
                # ACT is idle pre-first-exp; copy there to unblock S mms
                nc.scalar.copy(Yalt[m][:, 512 * jc:512 * (jc + 1)], ps[0:CE, 0:512])
            else:
                nc.vector.tensor_copy(Yalt[m][:, 512 * jc:512 * (jc + 1)],
                                      ps[0:CE, 0:512])
        for m in ("opt", "sar"):
            for jc in range(HALF // 512):
                emit_yalt(m, jc)

        v_pair = proj.tile([128, HW], dt.bfloat16)
        vT = proj.tile([128, HW // 128, 128], dt.bfloat16)
        # all 16 output blocks accumulate here; ONE tail DMA writes them out.
        # (per-block out-DMAs shrink Tile's DMA-semaphore-lane reuse window
        # and throttle the ACT queue through cross-queue alignment waits)
        obbuf = proj.tile([128, NBLK, C], dt.float32)

        def emit_fill(i, m, ci):
            cstart, clen = CHUNKS[ci]
            ps = ps_S.tile([128, 1536], dt.float32, tag="S", name="pss")
            for j in range(clen // 512):
                nc.tensor.matmul(ps[:, bass.ts(j, 512)],
                                 Yalt[m][:, 128 * i:128 * (i + 1)],
                                 x_ext[m][:, cstart + 512 * j:cstart + 512 * (j + 1)],
                                 start=True, stop=True)
            return ps

        def emit_exp(A, ps, ci):
            cstart, clen = CHUNKS[ci]
            nc.scalar.activation(A[:, cstart:cstart + clen], ps[:, 0:clen], AF.Exp)

        def emit_schrau(A, ps, ci):
            cstart, clen = CHUNKS[ci]
            nc.vector.tensor_scalar(
                A.bitcast(dt.uint16)[:, cstart:cstart + clen], ps[:, 0:clen],
                SCHRAU_SCALE, SCHRAU_BIAS,
                mybir.AluOpType.mult, mybir.AluOpType.add)

        def emit_pe_transpose_chunk(AT, A, ci):
            cstart, clen = CHUNKS[ci]
            t0 = cstart // 128
            nt = clen // 128
            done = 0
            while done < nt:
                gn = min(8, nt - done)
                pst = ps_T.tile([128, 1024], dt.bfloat16, tag="T", name="pst")
                for t in range(gn):
                    nc.tensor.transpose(pst[:, bass.ts(t, 128)],
                                        A[:, bass.ts(t0 + done + t, 128)], ident_bf[:])
                nc.vector.tensor_copy(AT[:, t0 + done:t0 + done + gn, :],
                                      pst[:, 0:gn * 128])
                done += gn

        def emit_dma_transpose(AT, A, ci):
            cstart, clen = CHUNKS[ci]
            nc.scalar.dma_start_transpose(
                AT[:, cstart // 128:(cstart + clen) // 128, :],
                A[:, cstart:cstart + clen])

        def alloc_block():
            A_o = workA.tile([128, HW], dt.bfloat16, tag="A_opt", name="A_o")
            A_s = workA.tile([128, HW], dt.bfloat16, tag="A_sar", name="A_s")
            AT_o = workT.tile([128, HW // 128, 128], dt.bfloat16, tag="AT_opt", name="AT_o")
            AT_s = workT.tile([128, HW // 128, 128], dt.bfloat16, tag="AT_sar", name="AT_s")
            PT = workP.tile([128, HW // 128, 128], dt.bfloat16, tag="PT", name="PT")
            # uz is allocated lazily at first z emission: the ps_U bank is
            # borrowed by v-projection staging during startup, and pool
            # rotation order must match use order to avoid deadlock.
            return dict(A_o=A_o, A_s=A_s, AT_o=AT_o, AT_s=AT_s, PT=PT, uz=None)

        def emit_pt_mult(blk, ci, pool=False):
            cstart, clen = CHUNKS[ci]
            lo, hi = cstart // 128, (cstart + clen) // 128
            eng = nc.gpsimd if pool else nc.vector
            eng.tensor_tensor(blk["PT"][:, lo:hi, :],
                              blk["AT_o"][:, lo:hi, :],
                              blk["AT_s"][:, lo:hi, :], mybir.AluOpType.mult)

        def emit_z(blk, which):
            AT = blk["AT_o"] if which == 0 else blk["AT_s"]
            if blk["uz"] is None:
                blk["uz"] = ps_U.tile([128, 130], dt.float32, tag="U", name="uz")
            uz = blk["uz"]
            for b in range(HW // 128):
                nc.tensor.matmul(uz[:, 128 + which:129 + which], AT[:, b, :],
                                 ones_bf[:],
                                 start=(b == 0), stop=(b == HW // 128 - 1))

        def emit_U(blk):
            uz = blk["uz"]
            PT = blk["PT"]
            for b in range(HW // 128):
                nc.tensor.matmul(uz[:, 0:128], PT[:, b, :], vT[:, b, :],
                                 start=(b == 0), stop=(b == HW // 128 - 1))

        def emit_out(i, blk):
            uz = blk["uz"]
            zsb = stats.tile([128, 2], dt.float32, tag="zsb")
            nc.vector.tensor_copy(zsb[:], uz[:, 128:130])
            denom = stats.tile([128, 1], dt.float32, tag="denom")
            nc.vector.tensor_mul(denom[:], zsb[:, 0:1], zsb[:, 1:2])
            inv = stats.tile([128, 1], dt.float32, tag="inv")
            nc.vector.reciprocal(inv[:], denom[:])
            inv2 = stats.tile([128, 1], dt.float32, tag="inv2")
            nc.vector.tensor_mul(inv2[:], inv[:], inv[:])
            t0 = stats.tile([128, C], dt.float32, tag="t0")
            nc.vector.tensor_scalar_mul(t0[:], uz[:, 0:C], inv2[:])
            nc.vector.tensor_mul(obbuf[:, i, :], t0[:], uz[:, C:128])

        _vstate = {"jc": 0}

        def emit_v_proj_chunks(n):
            # 512-col v-projection chunks, staged alternately through the
            # ps_T / ps_U banks (both idle until their first real use) so
            # matmul(c+1) overlaps the DVE copy of c.
            for _ in range(n):
                jc = _vstate["jc"]
                if jc >= 2 * (HW // 512):
                    return
                _vstate["jc"] += 1
                mi, col = divmod(jc, HW // 512)
                m = ("opt", "sar")[mi]
                if jc % 2 == 0:
                    psv = ps_T.tile([128, 512], dt.float32, tag="T", name="psvT")
                else:
                    psv = ps_U.tile([128, 512], dt.float32, tag="U", name="psvU")
                nc.tensor.matmul(psv[0:C, :], Lv[m][:],
                                 x_ext[m][:, bass.ts(col, 512)],
                                 start=True, stop=True)
                nc.vector.tensor_copy(v_pair[64 * mi:64 * mi + C, bass.ts(col, 512)],
                                      psv[0:C, :])

        def emit_v_transposes():
            for g in range(HW // 1024):
                pst = ps_T.tile([128, 1024], dt.bfloat16, tag="T", name="pstv")
                for t in range(8):
                    b = g * 8 + t
                    nc.tensor.transpose(pst[:, bass.ts(t, 128)],
                                        v_pair[:, bass.ts(b, 128)], ident_bf[:])
                nc.vector.tensor_copy(vT[:, g * 8:(g + 1) * 8, :], pst[:])

        # ---- startup: block 0 fills+exps, v-projection chunks interleaved
        blk = alloc_block()
        for m, A in (("opt", "A_o"), ("sar", "A_s")):
            for ci in range(3):
                ps = emit_fill(0, m, ci)
                if ci == 2:
                    emit_schrau(blk[A], ps, ci)
                else:
                    emit_exp(blk[A], ps, ci)
                emit_v_proj_chunks(3)
        emit_v_proj_chunks(99)
        emit_v_transposes()

        # ---- steady loop: iteration k finishes block k and emits block k+1.
        # DMA-transpose dispatches are placed ~3 exps after their source so
        # the ACT-SEQ-level engine-tick wait is always already satisfied.
        NT = NBLK * repeat
        for k in range(NT):
            kk = k % NBLK
            j = k + 1
            have_next = j < NT
            jj = j % NBLK
            nxt = alloc_block() if have_next else None

            # PE: the schrau-written c2 chunks MUST be PE-transposed: a
            # DMA-crossbar transpose on the ACT queue only serializes with
            # ACT-written sources; against a DVE producer it races (measured).
            if have_next:
                ps = emit_fill(jj, "opt", 0)
                emit_exp(nxt["A_o"], ps, 0)
            emit_pe_transpose_chunk(blk["AT_o"], blk["A_o"], 2)
            if have_next:
                ps = emit_fill(jj, "opt", 1)
                emit_exp(nxt["A_o"], ps, 1)
            emit_pe_transpose_chunk(blk["AT_s"], blk["A_s"], 2)

            # ACT queue: block k's DMA-transpose dispatches (ACT-written
            # chunks only); the SEQ trails block k+1's exps here, so block
            # k's engine ticks are retired and dispatch is instant.
            emit_dma_transpose(blk["AT_o"], blk["A_o"], 0)
            emit_dma_transpose(blk["AT_o"], blk["A_o"], 1)
            emit_dma_transpose(blk["AT_s"], blk["A_s"], 0)
            emit_dma_transpose(blk["AT_s"], blk["A_s"], 1)

            emit_pt_mult(blk, 0)
            emit_z(blk, 0)
            emit_pt_mult(blk, 1)
            emit_z(blk, 1)
            emit_pt_mult(blk, 2)
            if have_next:
                ps = emit_fill(jj, "sar", 0)
                emit_exp(nxt["A_s"], ps, 0)
                ps = emit_fill(jj, "sar", 1)
                emit_exp(nxt["A_s"], ps, 1)
                ps = emit_fill(jj, "opt", 2)
                emit_exp(nxt["A_o"], ps, 2)
                ps = emit_fill(jj, "sar", 2)
                emit_schrau(nxt["A_s"], ps, 2)
            emit_U(blk)
            emit_out(kk, blk)
            blk = nxt

        nc.sync.dma_start(d_out.rearrange("p (i c) -> p i c", c=C), obbuf[:])

    nc.compile()
    return nc


def _to_f32r(x):
    """Round fp32 to the float32r format: RNE to 11 mantissa bits, low 12 bits zero."""
    u = np.ascontiguousarray(x, np.float32).view(np.uint32)
    lsb = (u >> 12) & 1
    r = (u + np.uint32(0x7FF) + lsb) & np.uint32(0xFFFFF000)
    return r.view(np.float32)


def kernel(x_opt, x_sar, wq_opt, bq_opt, wk_opt, bk_opt, wv_opt, bv_opt,
           wq_sar, bq_sar, wk_sar, bk_sar, wv_sar, bv_sar, _trace=False):
    global _compiled
    if _compiled is None:
        _compiled = _build()
    nc = _compiled

    common = {}
    for m, wq, bq, wk, bk, wv, bv in (
            ("opt", wq_opt, bq_opt, wk_opt, bk_opt, wv_opt, bv_opt),
            ("sar", wq_sar, bq_sar, wk_sar, bk_sar, wv_sar, bv_sar)):
        wq = np.asarray(wq, np.float64); wk = np.asarray(wk, np.float64)
        bq = np.asarray(bq, np.float64); bk = np.asarray(bk, np.float64)
        Gm = np.zeros((CE, CE), np.float64)
        Gm[0:C, 0:C] = wq.T @ wk          # i-side rows, j-side cols
        Gm[C, 0:C] = wk.T @ bq            # column-linear term
        Gm[C, C] = float(bq @ bk) - SHIFT
        common[f"G_{m}"] = _to_f32r(Gm.astype(np.float32))
        Lvm = np.zeros((CE, C), np.float64)
        Lvm[0:C, :] = np.asarray(wv, np.float64).T
        Lvm[C, :] = np.asarray(bv, np.float64)
        common[f"Lv_{m}"] = _to_f32r(Lvm.astype(np.float32))

    in_maps = []
    for core in range(8):
        n, h = core // 2, core % 2
        mdict = dict(common)
        for m, x in (("opt", x_opt), ("sar", x_sar)):
            xs = np.asarray(x[n], np.float32).reshape(C, HW)
            if h:
                xs = np.concatenate([xs[:, HALF:], xs[:, :HALF]], axis=1)
            xs = np.concatenate([xs, np.ones((1, HW), np.float32)], axis=0)
            mdict[f"x_{m}"] = _to_f32r(xs)
        in_maps.append(mdict)

    r = run_bass_kernel_spmd(nc, in_maps, core_ids=list(range(8)), trace=_trace)
    out = np.empty((N, C, HW), np.float32)
    for core in range(8):
        n, h = core // 2, core % 2
        ob = r.results[core]["out"].reshape(128, NBLK, C)
        out[n][:, h * HALF:(h + 1) * HALF] = ob.transpose(1, 0, 2).reshape(HALF, C).T
    kernel._last_result = r
    return out.reshape(N, C, HW).reshape(N, C, H, W)
